# revision 46
# baseline (speedup 1.0000x reference)
# Trainium2 Bass kernel for nn_DecoderLayer (dense transformer decoder layer).
#
# Sharding: 8 cores = 2 batch groups (data-parallel over B=2) x 4-way
# sequence-shard of the 2048 rows (512 rows/core).  Each core projects K/V
# for ONLY its own 512 rows, then AllGathers K/V across its 4-core batch
# group (eliminating the 4x-redundant KV projection of the naive scheme).
# Each K/V gather is split into two halves so attention over heads 0-7 can
# start as soon as the first half lands; cross-attention K/V depend only on
# the encoder input X, so their projection + gather overlap fully with
# self-attention compute.  KV1 is projected first (it feeds the serial
# collective ring, the critical path); Q1 follows and fills the gather
# wait.
#
# Precision: all matmul operands are bf16 (PSUM accumulates fp32).  Softmax
# is max-free; the denominator rides the AV matmul as a ones-augmented V
# column; its reciprocal uses the fast DVE approximation (must read from
# SBUF - reading PSUM directly returns garbage on HW).  Exp runs on the
# scalar engine batched [128, 1024] over pairs of key blocks to amortize
# the ~350-cycle ACT instruction overhead; all other elementwise work
# (biases, relu, masks, LN affine) runs on the vector engine.  LayerNorm
# partition-axis reductions use ones-vector matmuls.  The LN2->FFN boundary
# is folded: FFN's first matmul consumes the pre-LN residual with
# host-folded weights A2 = wf1*g2 and is corrected by r and m*r broadcasts
# afterwards, so the 256 FFN matmuls overlap the LayerNorm chain; the
# LN1->Q2 boundary is folded the same way (A1 = wq2*g1).
#
# Big DMAs (weights) issue from the gpsimd queue; activations and stores
# from sync - keeping descriptor generation off any single hot queue.
import numpy as np
import ml_dtypes

import concourse.bacc as bacc
import concourse.bass as bass
import concourse.mybir as mybir
import concourse.tile as tile
from concourse.bass import ds
from concourse.bass_utils import run_bass_kernel_spmd

B, S, D, H, DK, F = 2, 2048, 1024, 16, 64, 4096
P = 128
NCORES = 8
GP = 4            # cores per batch group
R = S // GP       # rows per core (512)
DC = D // P       # 8 chunks of D
FC = F // P       # 32 chunks of F
SB = S // P       # 16 key blocks of 128
EPS = 1e-5
F32 = mybir.dt.float32
F32R = mybir.dt.float32r
BF16 = mybir.dt.bfloat16
AF = mybir.ActivationFunctionType
ALU = mybir.AluOpType

_TAPS = None  # set to a dict by debug harnesses to capture intermediates

WDD = ["wq1", "wk1", "wv1", "wo1", "wq2", "wk2", "wv2", "wo2"]
BDD = ["bq1", "bk1", "bv1", "bo1", "bq2", "bk2", "bv2", "bo2"]
LNP = ["g1", "be1", "g2", "be2", "g3", "be3"]
GROUPS = [[0, 1, 2, 3], [4, 5, 6, 7]]


def _bcast_ap(ap, parts):
    # [n] DRAM vector -> [parts, n] partition-broadcast AP
    return bass.AP(tensor=ap.tensor, offset=ap.offset,
                   ap=[[0, parts]] + list(ap.ap))


def _build_body(tc, io):
    nc = tc.nc

    # PSUM: psA 2x[128,512] + psS 2x[128,2,512] + psV 2x[65,512] = 16KB/part
    psA = tc.alloc_tile_pool(name="psA", bufs=2, space="PSUM")
    psS = tc.alloc_tile_pool(name="psS", bufs=2, space="PSUM")
    psV = tc.alloc_tile_pool(name="psV", bufs=2, space="PSUM")
    consts = tc.alloc_tile_pool(name="consts", bufs=1)
    persist = tc.alloc_tile_pool(name="persist", bufs=1)

    # ---- input activations first: the KV projections need them and the
    # sync queue should not be clogged by constant loads at t=0
    ysb = persist.tile([P, DC, R], BF16, name="ysb", tag="ysb")
    nc.sync.dma_start(out=ysb,
                      in_=io["yTo"].rearrange("(c p) r -> p c r", p=P))
    xsb = persist.tile([P, DC, R], BF16, name="xsb", tag="xsb")
    nc.sync.dma_start(out=xsb,
                      in_=io["xTo"].rearrange("(c p) r -> p c r", p=P))

    # ---- constants ------------------------------------------------------
    ones_f = consts.tile([P, P], F32)
    nc.vector.memset(ones_f, 1.0)
    ones_bf = consts.tile([P, 1], BF16)
    nc.vector.tensor_copy(out=ones_bf, in_=ones_f[:, 0:1])
    ones_row = consts.tile([1, P], F32R)
    nc.vector.tensor_copy(out=ones_row, in_=ones_f[0:1, :])
    ones_col = consts.tile([P, SB, 1], BF16)
    nc.vector.memset(ones_col, 1.0)
    bsb = {}
    for n in BDD + LNP + ["bf2"]:
        t = consts.tile([P, DC], F32, name=f"c_{n}", tag=f"c_{n}")
        nc.sync.dma_start(out=t, in_=io[n].rearrange("(c p) -> p c", p=P))
        bsb[n] = t
    bf1_sb = consts.tile([P, FC], F32)
    nc.scalar.dma_start(out=bf1_sb,
                        in_=io["bf1"].rearrange("(c p) -> p c", p=P))
    bvbc = {}
    for n in ["bv1", "bv2"]:
        tf = consts.tile([P, D], BF16, name=f"bc_{n}", tag=f"bc_{n}")
        nc.scalar.dma_start(out=tf, in_=_bcast_ap(io[f"{n}h"], P))
        bvbc[n] = tf

    # ---- persistent activations ----------------------------------------
    def act_tile(name, bufs=2):
        return persist.tile([P, DC, R], BF16, name=name, tag="act", bufs=bufs)

    # ---- KV projection + AllGather --------------------------------------
    ap_ = tc.alloc_tile_pool(name="attn", bufs=1)
    wkv = tc.alloc_tile_pool(name="wkv", bufs=1)

    def kv_proj_gather(blk, src_sb, wk_io, wv_io, bk_sb, bv_bc, grp):
        wk_sb = wkv.tile([P, DC, D], BF16, name=f"wk{blk}", tag="wdd", bufs=2)
        nc.gpsimd.dma_start(out=wk_sb,
                            in_=wk_io.rearrange("(c p) n -> p c n", p=P))
        wv_sb = wkv.tile([P, DC, D], BF16, name=f"wv{blk}", tag="wdd", bufs=2)
        nc.gpsimd.dma_start(out=wv_sb,
                            in_=wv_io.rearrange("(c p) n -> p c n", p=P))
        # halves (h) gather separately so attention t=0-3 can start early
        for h in range(2):
            bK = io[f"bK{blk}_{h}"]
            bV = io[f"bV{blk}_{h}"]
            # K^T (own rows): [dout half, R]
            for dh in range(DC // 2):
                do = 4 * h + dh
                ps = psA.tile([P, R], F32, tag="proj")
                for kc in range(DC):
                    nc.tensor.matmul(ps, wk_sb[:, kc, ds(P * do, P)],
                                     src_sb[:, kc, :],
                                     start=(kc == 0), stop=(kc == DC - 1))
                stg = wkv.tile([P, R], BF16, name="kstg", tag="kstg", bufs=2)
                nc.vector.tensor_scalar(stg, ps, bk_sb[:, do:do + 1], None,
                                        op0=ALU.add)
                nc.sync.dma_start(
                    out=bK.rearrange("(d r) -> d r", r=R)[ds(P * dh, P), :],
                    in_=stg)
            nc.gpsimd.collective_compute(
                "AllGather", ALU.bypass, replica_groups=grp,
                ins=[bK.opt()], outs=[io[f"gK{blk}_{h}"].opt()])
            # V (own rows): [R, dout half]
            for rb in range(R // P):
                ps = psA.tile([P, R], F32, tag="proj")
                for kc in range(DC):
                    nc.tensor.matmul(ps, src_sb[:, kc, ds(P * rb, P)],
                                     wv_sb[:, kc, ds(512 * h, 512)],
                                     start=(kc == 0), stop=(kc == DC - 1))
                stg = wkv.tile([P, R], BF16, name="vstg", tag="vstg", bufs=2)
                nc.vector.tensor_tensor(out=stg, in0=ps,
                                        in1=bv_bc[:, ds(512 * h, 512)],
                                        op=ALU.add)
                nc.sync.dma_start(
                    out=bV.rearrange("(r d) -> r d", d=D // 2)[
                        ds(P * rb, P), :],
                    in_=stg)
            nc.gpsimd.collective_compute(
                "AllGather", ALU.bypass, replica_groups=grp,
                ins=[bV.opt()], outs=[io[f"gV{blk}_{h}"].opt()])

    def q_proj(blk, qsrc, wq_io, bq_sb):
        qT = ap_.tile([P, DC, R], BF16, name=f"qT{blk}", tag="qT", bufs=2)
        for do in range(DC):
            wqs = ap_.tile([P, DC, P], BF16, name="wqs", tag="wcol", bufs=3)
            nc.gpsimd.dma_start(
                out=wqs,
                in_=wq_io[:, ds(P * do, P)].rearrange("(c p) n -> p c n", p=P))
            ps = psA.tile([P, R], F32, tag="proj")
            for kc in range(DC):
                nc.tensor.matmul(ps, wqs[:, kc, :], qsrc[:, kc, :],
                                 start=(kc == 0), stop=(kc == DC - 1))
            nc.vector.tensor_scalar(qT[:, do, :], ps, bq_sb[:, do:do + 1],
                                    None, op0=ALU.add)
        return qT


    grp = GROUPS
    kv_proj_gather(1, ysb, io["wk1"], io["wv1"], bsb["bk1"], bvbc["bv1"], grp)
    qT1 = q_proj(1, ysb, io["wq1"], bsb["bq1"])
    kv_proj_gather(2, xsb, io["wk2"], io["wv2"], bsb["bk2"], bvbc["bv2"], grp)

    def make_va_src(blk):
        def va_src(t, a):
            h, tl = t // 4, t % 4
            return io[f"gV{blk}_{h}"].rearrange(
                "c (rb p d) -> p (c rb) d", p=P, d=D // 2)[
                :, :, ds(P * tl + DK * a, DK)]
        return va_src

    # ---- shared attention-phase pools -----------------------------------
    mask_sb = persist.tile([P, SB, R], BF16, name="mask_sb", tag="mask")
    nc.sync.dma_start(out=mask_sb,
                      in_=io["mask"].rearrange("(kb p) q -> p kb q", p=P))

    def layernorm(x_sb, g_sb, be_sb, out_chunk_fn, export_rm=None):
        """LN over the partition(chunk) axis of x_sb [P, DC, R] (bf16)."""
        red = psS.tile([P, 2, R], F32, tag="sc", name="lnred")
        ps_sum = red[0:1, 0, :]
        ps_sq = red[0:1, 1, :]
        for c in range(DC):
            sqc = ap_.tile([P, R], BF16, name="sqc", tag="sqc", bufs=2)
            nc.vector.tensor_mul(sqc, x_sb[:, c, :], x_sb[:, c, :])
            nc.tensor.matmul(ps_sum, ones_bf, x_sb[:, c, :],
                             start=(c == 0), stop=(c == DC - 1))
            nc.tensor.matmul(ps_sq, ones_bf, sqc,
                             start=(c == 0), stop=(c == DC - 1))
        mean = ap_.tile([1, R], F32R, name="mean", tag="st", bufs=4)
        nc.scalar.mul(mean, ps_sum, 1.0 / D)
        msq = ap_.tile([1, R], F32, name="msq", tag="st", bufs=4)
        nc.vector.tensor_mul(msq, mean, ps_sum)          # sum^2 / D
        varn = ap_.tile([1, R], F32, name="varn", tag="st", bufs=4)
        nc.vector.tensor_sub(varn, ps_sq, msq)
        sd = ap_.tile([1, R], F32, name="sd", tag="st", bufs=4)
        nc.scalar.activation(sd, varn, AF.Sqrt, scale=1.0 / (D - 1))
        nc.vector.tensor_scalar_add(sd, sd, EPS)
        rr = ap_.tile([1, R], F32, name="rr", tag="st", bufs=4)
        nc.vector.reciprocal_approx_fast(rr, sd)
        rrr = ap_.tile([1, R], F32R, name="rrr", tag="st2", bufs=2)
        with nc.allow_low_precision(reason="f32r same width as f32"):
            nc.vector.tensor_copy(out=rrr, in_=rr)
        # broadcast mean/rstd across partitions via K=1 outer-product matmuls
        bcast = psS.tile([P, 2, R], F32, tag="sc", name="lnbc")
        mb = bcast[:, 0, :]
        rb = bcast[:, 1, :]
        nc.tensor.matmul(mb, ones_row, mean, start=True, stop=True)
        nc.tensor.matmul(rb, ones_row, rrr, start=True, stop=True)
        if export_rm is not None:
            rb_sb, mrb_sb = export_rm
            with nc.allow_low_precision(reason="f32r same width as f32"):
                nc.vector.tensor_copy(out=rb_sb, in_=rb)
                nc.vector.tensor_mul(mrb_sb, mb, rb_sb)
        for c in range(DC):
            t = ap_.tile([P, R], F32R, name="lnt", tag="sqc", bufs=2)
            nc.vector.tensor_sub(t, x_sb[:, c, :], mb)
            nc.vector.tensor_mul(t, t, rb)
            out_chunk_fn(c, t, g_sb[:, c:c + 1], be_sb[:, c:c + 1])

    def q_proj_folded(res_sb, rb_sb, mrb_sb, a_io, un_sb, vq_sb):
        """Q2 on the pre-LN residual: q = r*(A@res) - (m*r)*u + v."""
        qT = ap_.tile([P, DC, R], BF16, name="qT2", tag="qT", bufs=2)
        for do in range(DC):
            wqs = ap_.tile([P, DC, P], BF16, name="wqs", tag="wcol", bufs=3)
            nc.gpsimd.dma_start(
                out=wqs,
                in_=a_io[:, ds(P * do, P)].rearrange("(c p) n -> p c n", p=P))
            ps = psA.tile([P, R], F32, tag="proj")
            for kc in range(DC):
                nc.tensor.matmul(ps, wqs[:, kc, :], res_sb[:, kc, :],
                                 start=(kc == 0), stop=(kc == DC - 1))
            t1 = ap_.tile([P, R], F32R, name="qt1", tag="qt1", bufs=2)
            nc.vector.tensor_mul(t1, ps, rb_sb)
            with nc.allow_low_precision(reason="f32r same width as f32"):
                nc.vector.scalar_tensor_tensor(
                    out=t1, in0=mrb_sb, scalar=un_sb[:, do:do + 1],
                    in1=t1, op0=ALU.mult, op1=ALU.add)
            nc.vector.tensor_scalar(qT[:, do, :], t1, vq_sb[:, do:do + 1],
                                    None, op0=ALU.add)
        return qT

    def attn_block(blk, qT, qsrc, wo_io, bo_sb,
                   masked, g_sb, be_sb, kst=None, va_src=None,
                   export_rm=None):
        """One attention sublayer; returns post-LN [P, DC, R] bf16 tile."""
        attT = ap_.tile([P, DC, R], BF16, name=f"attT{blk}", tag="attT",
                        bufs=1)

        for t in range(H // 2):
            if kst is None:
                h, tl = t // 4, t % 4
                gK = io[f"gK{blk}_{h}"]
                # khp [P, GP, R]: key k = c*R + r lives at [:, c, r]
                khp = ap_.tile([P, GP, R], BF16, name="khp", tag="khp",
                               bufs=2)
                nc.sync.dma_start(
                    out=khp,
                    in_=gK[:, ds(P * tl * R, P * R)].rearrange(
                        "c (p r) -> p c r", p=P))
                kslc = (lambda khp_: lambda a, kb: khp_[
                    ds(DK * a, DK), kb // 4, ds((kb % 4) * P, P)])(khp)
            else:
                kslc = lambda a, kb, t_=t: kst(t_, a, kb)
            vab = []
            for a in range(2):
                va = ap_.tile([P, SB, DK + 1], BF16, name=f"va{a}",
                              tag="vaug", bufs=4)
                nc.sync.dma_start(out=va[:, :, 0:DK], in_=va_src(t, a))
                nc.vector.tensor_copy(out=va[:, :, DK:DK + 1], in_=ones_col)
                vab.append(va)
            pv = [psV.tile([DK + 1, R], F32, tag="av", name=f"pv{a}")
                  for a in range(2)]
            for kp in range(SB // 2):
                scf = [psS.tile([P, 2, R], F32, tag="sc", name=f"sc{a}")
                       for a in range(2)]
                sc = [[scf[a][:, j, :] for j in range(2)] for a in range(2)]
                for j in range(2):
                    kb = 2 * kp + j
                    for a in range(2):
                        nc.tensor.matmul(
                            sc[a][j],
                            kslc(a, kb),
                            qT[ds(DK * a, DK), t, :],
                            start=True, stop=True,
                            tile_position=(DK * a, 0))
                exb = []
                for a in range(2):
                    ex = ap_.tile([P, 2, R], BF16, name="ex", tag="exp",
                                  bufs=3)
                    nc.scalar.activation(ex, scf[a], AF.Exp, scale=0.125)
                    if masked:
                        nc.vector.tensor_mul(
                            ex, ex, mask_sb[:, ds(2 * kp, 2), :])
                    exb.append(ex)
                for j in range(2):
                    for a in range(2):
                        nc.tensor.matmul(
                            pv[a], vab[a][:, 2 * kp + j, :], exb[a][:, j, :],
                            start=(kp == 0 and j == 0),
                            stop=(kp == SB // 2 - 1 and j == 1))
            for a in range(2):
                den = ap_.tile([1, R], F32, name="den", tag="den", bufs=1)
                nc.vector.tensor_copy(out=den, in_=pv[a][DK:DK + 1, :])
                rc = ap_.tile([1, R], F32, name="rc", tag="rc", bufs=1)
                nc.vector.reciprocal_approx_fast(rc, den)
                rcr = ap_.tile([1, R], F32R, name="rcr", tag="rcr", bufs=1)
                with nc.allow_low_precision(reason="f32r same width as f32"):
                    nc.vector.tensor_copy(out=rcr, in_=rc)
                rcb = psA.tile([P, R], F32, name="rcb", tag="proj")
                nc.tensor.matmul(rcb[:DK], ones_row[:, :DK], rcr,
                                 start=True, stop=True)
                rcs = ap_.tile([DK, R], F32R, name="rcs", tag="rcs", bufs=2)
                nc.vector.tensor_copy(out=rcs, in_=rcb[:DK])
                nc.vector.tensor_mul(attT[ds(DK * a, DK), t, :],
                                     pv[a][0:DK, :], rcs)

        # output projection + bias + residual, then LN
        res_sb = act_tile(f"res{blk}")
        for do in range(DC):
            wos = ap_.tile([P, DC, P], BF16, name="wos", tag="wcol", bufs=3)
            nc.gpsimd.dma_start(
                out=wos,
                in_=wo_io[:, ds(P * do, P)].rearrange("(c p) n -> p c n", p=P))
            ps = psA.tile([P, R], F32, tag="proj")
            for kc in range(DC):
                nc.tensor.matmul(ps, wos[:, kc, :], attT[:, kc, :],
                                 start=(kc == 0), stop=(kc == DC - 1))
            nc.vector.scalar_tensor_tensor(
                out=res_sb[:, do, :], in0=ps, scalar=bo_sb[:, do:do + 1],
                in1=qsrc[:, do, :], op0=ALU.add, op1=ALU.add)
        out_sb = act_tile(f"ln{blk}")

        def _emit(c, t_, g, be):
            nc.vector.tensor_scalar(out_sb[:, c, :], t_, g, be,
                                    op0=ALU.mult, op1=ALU.add)
        layernorm(res_sb, g_sb, be_sb, _emit, export_rm=export_rm)
        for nm, t in ((f"dbg_qT{blk}", qT), (f"dbg_attT{blk}", attT),
                      (f"dbg_ln{blk}", out_sb)):
            if nm in io:
                nc.sync.dma_start(
                    out=io[nm].rearrange("(c p) r -> p c r", p=P), in_=t)
        return out_sb, res_sb

    rb1_sb = persist.tile([P, R], F32R, name="rb1_sb", tag="rb1")
    mrb1_sb = persist.tile([P, R], F32, name="mrb1_sb", tag="mrb1")
    u1_sb = consts.tile([P, DC], F32, name="u1_sb", tag="u1")
    nc.gpsimd.dma_start(out=u1_sb,
                        in_=io["u1n"].rearrange("(c p) -> p c", p=P))
    v1_sb = consts.tile([P, DC], F32, name="v1_sb", tag="v1")
    nc.gpsimd.dma_start(out=v1_sb,
                        in_=io["v1q"].rearrange("(c p) -> p c", p=P))
    out1, res1 = attn_block(1, qT1, ysb, io["wo1"],
                            bsb["bo1"], True, bsb["g1"], bsb["be1"],
                            va_src=make_va_src(1),
                            export_rm=(rb1_sb, mrb1_sb))
    qT2 = q_proj_folded(res1, rb1_sb, mrb1_sb, io["A1"], u1_sb, v1_sb)
    rb2_sb = persist.tile([P, R], F32R, name="rb2_sb", tag="rb2")
    mrb2_sb = persist.tile([P, R], F32, name="mrb2_sb", tag="mrb2")
    out2, res2 = attn_block(2, qT2, out1, io["wo2"],
                            bsb["bo2"], False, bsb["g2"], bsb["be2"],
                            va_src=make_va_src(2),
                            export_rm=(rb2_sb, mrb2_sb))
    wkv.release()

    # ---- FFN -------------------------------------------------------------
    fp = tc.alloc_tile_pool(name="ffn", bufs=1)
    hT = fp.tile([P, FC, R], BF16, name="hT", tag="hT")
    u2_sb = fp.tile([P, FC], F32, name="u2_sb", tag="u2")
    nc.gpsimd.dma_start(out=u2_sb,
                        in_=io["u2n"].rearrange("(c p) -> p c", p=P))
    v2_sb = fp.tile([P, FC], F32, name="v2_sb", tag="v2")
    nc.gpsimd.dma_start(out=v2_sb,
                        in_=io["v2b"].rearrange("(c p) -> p c", p=P))
    for g in range(16):
        wf1g = fp.tile([P, DC, 256], BF16, name="wf1g", tag="wf1g", bufs=2)
        nc.gpsimd.dma_start(
            out=wf1g,
            in_=io["A2"][:, ds(256 * g, 256)].rearrange(
                "(c p) n -> p c n", p=P))
        for fo in range(2):
            f = 2 * g + fo
            ps = psA.tile([P, R], F32, tag="proj")
            for kc in range(DC):
                nc.tensor.matmul(ps, wf1g[:, kc, ds(P * fo, P)],
                                 res2[:, kc, :],
                                 start=(kc == 0), stop=(kc == DC - 1))
            # z = r*(A2@res2) - (m*r)*u2 + v2b ; h = relu(z)
            t1 = fp.tile([P, R], F32R, name="ft1", tag="ft1", bufs=2)
            nc.vector.tensor_mul(t1, ps, rb2_sb)
            with nc.allow_low_precision(reason="f32r same width as f32"):
                nc.vector.scalar_tensor_tensor(
                    out=t1, in0=mrb2_sb, scalar=u2_sb[:, f:f + 1],
                    in1=t1, op0=ALU.mult, op1=ALU.add)
            nc.vector.tensor_scalar(hT[:, f, :], t1, v2_sb[:, f:f + 1],
                                    0.0, op0=ALU.add, op1=ALU.max)
    fT = act_tile("fT")
    for do in range(DC):
        wf2s = fp.tile([P, FC, P], BF16, name="wf2s", tag="wf2s", bufs=2)
        nc.gpsimd.dma_start(
            out=wf2s,
            in_=io["wf2"][:, ds(P * do, P)].rearrange("(c p) n -> p c n", p=P))
        ps = psA.tile([P, R], F32, tag="proj")
        for fc in range(FC):
            nc.tensor.matmul(ps, wf2s[:, fc, :], hT[:, fc, :],
                             start=(fc == 0), stop=(fc == FC - 1))
        nc.vector.scalar_tensor_tensor(
            out=fT[:, do, :], in0=ps, scalar=bsb["bf2"][:, do:do + 1],
            in1=out2[:, do, :], op0=ALU.add, op1=ALU.add)

    def _emit_out(c, t_, g, be):
        stg = ap_.tile([P, R], F32, name="ostg", tag="ostg", bufs=1)
        nc.vector.tensor_scalar(stg, t_, g, be, op0=ALU.mult, op1=ALU.add)
        nc.sync.dma_start(out=io["outT"][ds(P * c, P), :], in_=stg)
    layernorm(fT, bsb["g3"], bsb["be3"], _emit_out)
    fp.release()

    ap_.release()
    persist.release()
    consts.release()
    for p in (psV, psS, psA):
        p.release()


def build_nc():
    nc = bacc.Bacc("TRN2", target_bir_lowering=False, debug=False,
                   num_devices=NCORES)
    io = {}

    def inp(name, shape, dtype=BF16):
        io[name] = nc.dram_tensor(name, shape, dtype,
                                  kind="ExternalInput").ap()

    inp("yTo", [D, R])
    inp("xTo", [D, R])
    inp("mask", [S, R])
    for n in WDD:
        inp(n, [D, D])
    inp("A1", [D, D])
    inp("A2", [D, F])
    inp("wf2", [F, D])
    for n in BDD + LNP + ["bf2"]:
        inp(n, [D], F32)
    inp("bv1h", [D])
    inp("bv2h", [D])
    inp("bf1", [F], F32)
    inp("u2n", [F], F32)
    inp("v2b", [F], F32)
    inp("u1n", [D], F32)
    inp("v1q", [D], F32)
    io["outT"] = nc.dram_tensor("outT", [D, R], F32,
                                kind="ExternalOutput").ap()
    hsz = D * R // 2
    for blk in (1, 2):
        for h in range(2):
            io[f"bK{blk}_{h}"] = nc.dram_tensor(
                f"bK{blk}_{h}", [hsz], BF16).ap()
            io[f"bV{blk}_{h}"] = nc.dram_tensor(
                f"bV{blk}_{h}", [hsz], BF16).ap()
            io[f"gK{blk}_{h}"] = nc.dram_tensor(
                f"gK{blk}_{h}", [GP, hsz], BF16).ap()
            io[f"gV{blk}_{h}"] = nc.dram_tensor(
                f"gV{blk}_{h}", [GP, hsz], BF16).ap()
    with tile.TileContext(nc) as tc:
        _build_body(tc, io)
    nc.compile()
    return nc


_NC = None


def _get_nc():
    global _NC
    if _NC is None:
        _NC = build_nc()
    return _NC


def make_in_maps(inputs):
    bf = ml_dtypes.bfloat16
    gi = {k: np.asarray(v) for k, v in inputs.items()}
    shared = {n: np.ascontiguousarray(gi[n].astype(bf))
              for n in WDD + ["wf2"]}
    a1 = gi["wq2"].astype(np.float32) * gi["g1"].astype(np.float32)[:, None]
    shared["A1"] = np.ascontiguousarray(a1.astype(bf))
    shared["u1n"] = np.ascontiguousarray(-a1.sum(axis=0).astype(np.float32))
    v1q = (gi["wq2"].astype(np.float32)
           * gi["be1"].astype(np.float32)[:, None]).sum(axis=0)
    shared["v1q"] = np.ascontiguousarray(
        (v1q + gi["bq2"].astype(np.float32)).astype(np.float32))
    a2 = gi["wf1"].astype(np.float32) * gi["g2"].astype(np.float32)[:, None]
    shared["A2"] = np.ascontiguousarray(a2.astype(bf))
    shared["u2n"] = np.ascontiguousarray(-a2.sum(axis=0).astype(np.float32))
    v2b = (gi["wf1"].astype(np.float32)
           * gi["be2"].astype(np.float32)[:, None]).sum(axis=0)
    shared["v2b"] = np.ascontiguousarray(
        (v2b + gi["bf1"].astype(np.float32)).astype(np.float32))
    for n in BDD + LNP + ["bf1", "bf2"]:
        shared[n] = np.ascontiguousarray(gi[n].astype(np.float32))
    in_maps = []
    for c in range(NCORES):
        b, r0 = c // GP, (c % GP) * R
        mask = (np.arange(S)[:, None] <= (r0 + np.arange(R))[None, :])
        in_maps.append(dict(
            bv1h=gi["bv1"].astype(bf), bv2h=gi["bv2"].astype(bf),
            yTo=np.ascontiguousarray(gi["y"][b, r0:r0 + R].T.astype(bf)),
            xTo=np.ascontiguousarray(gi["X"][b, r0:r0 + R].T.astype(bf)),
            mask=np.ascontiguousarray(mask.astype(bf)),
            **shared))
    return in_maps


def kernel(**inputs):
    nc = _get_nc()
    in_maps = make_in_maps(inputs)
    res = run_bass_kernel_spmd(nc, in_maps, core_ids=list(range(NCORES)))
    out = np.empty((B, S, D), np.float32)
    for c in range(NCORES):
        out[c // GP, (c % GP) * R:(c % GP + 1) * R, :] = \
            res.results[c]["outT"].T
    return out
